# revision 14
# baseline (speedup 1.0000x reference)
"""Trainium2 Bass kernel for nn_HeatmapLayer: separable Gaussian heatmaps.

Reference math (per batch b, class c):
    mx = labels[b, 2c] * H ; my = labels[b, 2c+1] * W          (H = W = 384)
    sigma = H * exp(log_weight)
    dx2[h] = (h - mx)^2 / sigma        ; normalized by its min over h
    dy2[w] = (w - my)^2 / (20 * sigma) ; normalized by its min over w
    out[b,c,h,w] = exp(-0.5*(dx2[h] + dy2[w])) = ex[h] * ey[w]

Each (b,c) heatmap is a rank-1 outer product of two 384-length
profiles; 2 batches x 6 classes = 12 pairs per core (batch-parallel
over 8 cores).  The kernel is output-DMA-bound: 7.08MB/core over 16
DMA engines = ~17us of saturated drain, so everything else is
organized to start that drain as early as possible and keep it
gap-free.

Key structure:
  * Fused profile pipeline: y-side rows live on partitions 0..11 and
    x-side rows on partitions 32..43 of ONE [44, 384] tile, so a
    single ACT Square and a single ACT Exp (bf16 out) produce BOTH
    profiles; partition bases 0/32 satisfy the PE tile-position
    alignment for the matmuls (rhs base 0) and transposes (base 32).
  * PE outer-product: the idle PE engine broadcasts ey to all 128
    partitions (one K=12 matmul per pair against a 0/1
    block-selector); each output chunk is then exactly ONE DVE/ACT op
    reading PSUM and scaling by the transposed x-profile EXT
    (per-partition scalar) -- the engine minimum per output element.
  * The start-of-kernel all-engine barrier is rebuilt WITHOUT the
    GpSimd/Pool engine (nothing here uses it; constants arrive as
    host-provided DMA inputs and no float activation biases means no
    const-AP reads, the only thing the stock barrier protects here).
  * ACT's Exp table (~1.3us load) is warmed by a dummy Exp on a
    memset tile before the inputs even arrive.
  * The per-axis min of (h-m)^2 is computed EXACTLY from the labels
    alone with the +-2^23 round-to-integer trick + clamp (tiny DVE
    ops hidden under the ACT Square) instead of a 384-wide reduce.
  * Pair 0's three chunks stream out as separate DMAs right behind
    their finals (all on DVE, so the first DMA's semaphore wait is
    single-engine); later pairs go as whole 576KB DMAs.
  * Output rows are staged as h = 3*par + c so each SBUF partition is
    one contiguous 4608B DRAM run.  EXT[par,c,p] = ex_p[3*par+c] comes
    from 3 stride-3 PE transposes (bf16, single-pass).
"""

import numpy as np
from contextlib import ExitStack

import concourse.bacc as bacc
import concourse.bass as bass
import concourse.tile as tile
from concourse import mybir
from concourse.bass_utils import run_bass_kernel_spmd

B, CH, H, W = 16, 3, 384, 384
NCLS = 6
N_CORES = 8
BPC = B // N_CORES            # batches per core = 2
PAIRS = BPC * NCLS            # (b,c) pairs per core = 12
P = 128
C3 = H // P                   # 3 chunks of 128 rows
XROW = 32                     # partition base of the x-side rows
NROWS = XROW + PAIRS          # 44
LN_H = float(np.log(H))
RND = 12582912.0              # 1.5 * 2^23: add+subtract rounds to integer
F32 = mybir.dt.float32
BF16 = mybir.dt.bfloat16
AF = mybir.ActivationFunctionType
ALU = mybir.AluOpType

# packed f32 input [44, 386]: col0 = m-coord (y rows 0..11 get my,
# x rows 32..43 get mx, rows 12..31 zero), col1 = log_weight,
# cols 2..385 = j/384 grid.
PK_M, PK_LW, PK_GRID = 0, 1, 2
PK_N = PK_GRID + W
# packed bf16 input [44, 1548]: rows 0..11 cols 0..1535 = block
# selector (row k of block p is 1 iff k==p); rows 32..43 cols
# 1536..1547 = 12x12 identity.
SEL_ID = PAIRS * P
SEL_N = SEL_ID + PAIRS

# engine for the finals of pairs 1..11 ('vvv' is hardcoded for pair 0):
# DVE 18 / ACT 18 overall.
BALANCE = ["vvv"] + ["ava", "vav"] * 4 + ["ava"] * 3


def _barrier_without_pool(self, *, sem_only: bool = False):
    engines = [e for e in self.engines if e != mybir.EngineType.Pool]
    if sem_only:
        for inst in self._sem_only_all_engine_barrier_insts("aeb"):
            self.engines[inst.engine].add_instruction(inst)
    else:
        self.multi_engine_barrier(engines)


def build_bass() -> bass.Bass:
    orig_barrier = bass.Bass.all_engine_barrier
    bass.Bass.all_engine_barrier = _barrier_without_pool
    try:
        nc = bacc.Bacc("TRN2", target_bir_lowering=False, debug=False,
                       num_devices=N_CORES)
        _build_body(nc)
        nc.finalize()
    finally:
        bass.Bass.all_engine_barrier = orig_barrier
    return nc


def _build_body(nc) -> None:
    pack1 = nc.dram_tensor("pack1", [NROWS, PK_N], F32, kind="ExternalInput")
    pack2 = nc.dram_tensor("pack2", [NROWS, SEL_N], BF16,
                           kind="ExternalInput")
    out = nc.dram_tensor("out", [PAIRS * H, W], F32, kind="ExternalOutput")

    with ExitStack() as ctx:
        tc = ctx.enter_context(tile.TileContext(nc))
        singles = ctx.enter_context(tc.tile_pool(name="singles", bufs=1))
        psT = ctx.enter_context(tc.tile_pool(name="psT", bufs=2,
                                             space="PSUM"))
        psB = ctx.enter_context(tc.tile_pool(name="psB", bufs=6,
                                             space="PSUM"))
        stage = ctx.enter_context(tc.tile_pool(name="stage", bufs=12))

        # ---- input DMAs, one per HWDGE queue ------------------------------
        pk = singles.tile([NROWS, PK_N], F32)
        nc.sync.dma_start(out=pk, in_=pack1[:, :])
        sel = singles.tile([NROWS, SEL_N], BF16)
        nc.scalar.dma_start(out=sel, in_=pack2[:, :])

        mcol = pk[:, PK_M:PK_M + 1]
        lwcol = pk[:, PK_LW:PK_LW + 1]
        grid = pk[:, PK_GRID:PK_GRID + W]
        ident = sel[XROW:NROWS, SEL_ID:SEL_ID + PAIRS]

        # ---- warm the ACT Exp table before inputs arrive ------------------
        zz = singles.tile([NROWS, 2], F32)
        nc.vector.memset(zz, 0.0)
        warm = singles.tile([NROWS, 1], F32)
        nc.scalar.activation(out=warm, in_=zz[:, 0:1], func=AF.Exp,
                             bias=zz[:, 1:2], scale=1.0)
        zcol = zz[:, 1:2]                        # zeros bias AP

        # ---- per-row scalars (DVE, tiny; hidden under the ACT Square) -----
        # inv_s = exp(-lw - lnH); grid is j/384 so sq=(m-j/384)^2 and the
        # 384^2 folds into the exp scales (y: 0.025, x: 0.5).
        nlw = singles.tile([NROWS, 1], F32)
        nc.vector.tensor_scalar(out=nlw, in0=lwcol, scalar1=-1.0,
                                scalar2=-LN_H, op0=ALU.mult, op1=ALU.add)
        inv_s = singles.tile([NROWS, 1], F32)
        nc.scalar.activation(out=inv_s, in_=nlw, func=AF.Exp,
                             bias=zcol, scale=1.0)
        scpo = singles.tile([NROWS, 2], F32)    # col0: exp scale, col1: |.|
        nc.vector.memset(scpo, 0.0)
        HH = float(H) * float(H)
        nc.vector.tensor_scalar_mul(out=scpo[:PAIRS, 0:1],
                                    in0=inv_s[:PAIRS, :],
                                    scalar1=-0.025 * HH)
        nc.vector.tensor_scalar_mul(out=scpo[XROW:, 0:1], in0=inv_s[XROW:, :],
                                    scalar1=-0.5 * HH)
        nc.vector.tensor_scalar_mul(out=scpo[:PAIRS, 1:2],
                                    in0=inv_s[:PAIRS, :], scalar1=0.025)
        nc.vector.tensor_scalar_mul(out=scpo[XROW:, 1:2], in0=inv_s[XROW:, :],
                                    scalar1=0.5)

        # exact min of (h-m)^2 over integer h in [0,383], from labels only:
        # h* = clamp(round(384*m), max 383), min = (384*m-h*)^2 (grid^2)
        m2c = singles.tile([NROWS, 1], F32)
        nc.vector.tensor_scalar_mul(out=m2c, in0=mcol, scalar1=float(H))
        t1 = singles.tile([NROWS, 1], F32)
        nc.vector.tensor_scalar_add(out=t1, in0=m2c, scalar1=RND)
        rr = singles.tile([NROWS, 1], F32)      # round(m) (half-to-even)
        nc.vector.tensor_scalar_add(out=rr, in0=t1, scalar1=-RND)
        rc = singles.tile([NROWS, 1], F32)      # clamp to grid max
        nc.vector.tensor_scalar_min(out=rc, in0=rr, scalar1=float(H - 1))
        dd = singles.tile([NROWS, 1], F32)
        nc.vector.tensor_sub(out=dd, in0=m2c, in1=rc)
        mn = singles.tile([NROWS, 1], F32)
        nc.vector.tensor_mul(out=mn, in0=dd, in1=dd)
        nb = singles.tile([NROWS, 1], F32)      # exp bias: |sc|*min >= 0
        nc.vector.tensor_mul(out=nb, in0=mn, in1=scpo[:, 1:2])

        # ---- both profiles in one Square + one Exp ------------------------
        sq = singles.tile([NROWS, W], F32)
        nc.scalar.activation(out=sq, in_=grid, func=AF.Square,
                             bias=mcol, scale=-1.0)
        exy = singles.tile([NROWS, W], BF16)
        nc.scalar.activation(out=exy, in_=sq, func=AF.Exp,
                             bias=nb, scale=scpo[:, 0:1])
        ey = exy[:PAIRS, :]                     # y profiles (matmul rhs)

        # ---- first matmul + transposes unlock together --------------------
        def pair_matmul(p):
            ps = psB.tile([P, W], F32)
            nc.tensor.matmul(ps, sel[:PAIRS, p * P:(p + 1) * P], ey,
                             start=True, stop=True)
            return ps

        ps0 = pair_matmul(0)

        # EXT[par, c, p] = ex_p[3*par + c] via 3 strided bf16 PE transposes
        exr = exy[XROW:NROWS, :].rearrange("p (h c) -> p c h", c=C3)
        ext = singles.tile([P, C3, PAIRS], F32)
        for c in range(C3):
            pt = psT.tile([P, PAIRS], BF16)
            nc.tensor.transpose(pt, exr[:, c, :], ident)
            nc.vector.tensor_copy(out=ext[:, c, :], in_=pt)

        # ---- main loop: one final op per output chunk, then one DMA -------
        for p in range(PAIRS):
            ps = ps0 if p == 0 else pair_matmul(p)
            st = stage.tile([P, C3, W], F32)
            od = out[p * H:(p + 1) * H, :].rearrange(
                "(par c) w -> par c w", c=C3)
            for c in range(C3):
                scal = ext[:, c, p:p + 1]
                if BALANCE[p][c] == "v":
                    nc.vector.tensor_scalar_mul(out=st[:, c, :], in0=ps,
                                                scalar1=scal)
                else:
                    nc.scalar.mul(out=st[:, c, :], in_=ps, mul=scal)
                if p == 0:
                    nc.sync.dma_start(out=od[:, c, :], in_=st[:, c, :])
            # DRAM row (within pair p) = 3*par + c: one contiguous 4608B
            # run per partition.
            if p > 0:
                nc.sync.dma_start(out=od, in_=st)


LAST_RESULTS = None  # BassKernelResults of the most recent kernel() call


def _pack_inputs(labels: np.ndarray, log_weight: np.ndarray) -> np.ndarray:
    """[44, 386] per-core f32 pack: m-coords | logw | grid."""
    pk = np.zeros((NROWS, PK_N), dtype=np.float32)
    pk[:PAIRS, PK_M] = labels[:, 1]             # my on rows 0..11
    pk[XROW:, PK_M] = labels[:, 0]              # mx on rows 32..43
    pk[:, PK_LW] = np.float32(log_weight).reshape(())
    pk[:, PK_GRID:] = (np.arange(W, dtype=np.float32)
                       / np.float32(W))[None, :]
    return pk


def kernel(x: np.ndarray, labels: np.ndarray,
           log_weight: np.ndarray, **run_kwargs) -> np.ndarray:
    global LAST_RESULTS
    del x  # only its (hardcoded) shape matters
    import ml_dtypes
    nc = build_bass()
    labels = np.ascontiguousarray(labels, dtype=np.float32)
    sel = np.zeros((NROWS, SEL_N), dtype=np.float32)
    sel[:PAIRS, :SEL_ID] = np.kron(np.eye(PAIRS, dtype=np.float32),
                                   np.ones((1, P), dtype=np.float32))
    sel[XROW:, SEL_ID:] = np.eye(PAIRS, dtype=np.float32)
    sel = sel.astype(ml_dtypes.bfloat16)
    in_maps = [
        {
            "pack1": _pack_inputs(
                labels[i * BPC:(i + 1) * BPC].reshape(PAIRS, 2), log_weight),
            "pack2": sel,
        }
        for i in range(N_CORES)
    ]
    res = run_bass_kernel_spmd(nc, in_maps, core_ids=list(range(N_CORES)),
                               **run_kwargs)
    LAST_RESULTS = res
    outs = [r["out"].reshape(BPC, NCLS, H, W) for r in res.results]
    return np.concatenate(outs, axis=0)


if __name__ == "__main__":
    rng = np.random.default_rng(0)
    x = rng.standard_normal((B, CH, H, W), dtype=np.float32)
    labels = rng.random((B, 2 * NCLS), dtype=np.float32)
    lw = rng.random((1, 1, 1, 1), dtype=np.float32)
    y = kernel(x=x, labels=labels, log_weight=lw)
    print(y.shape, y.dtype, y.min(), y.max())


# revision 15
# speedup vs baseline: 1.1075x; 1.1075x over previous
"""Trainium2 Bass kernel for nn_HeatmapLayer: separable Gaussian heatmaps.

Reference math (per batch b, class c):
    mx = labels[b, 2c] * H ; my = labels[b, 2c+1] * W          (H = W = 384)
    sigma = H * exp(log_weight)
    dx2[h] = (h - mx)^2 / sigma        ; normalized by its min over h
    dy2[w] = (w - my)^2 / (20 * sigma) ; normalized by its min over w
    out[b,c,h,w] = exp(-0.5*(dx2[h] + dy2[w])) = ex[h] * ey[w]

Each (b,c) heatmap is a rank-1 outer product of two 384-length
profiles; 2 batches x 6 classes = 12 pairs per core (batch-parallel
over 8 cores).  The kernel is output-DMA-bound: 7.08MB/core over 16
DMA engines at ~360GB/s aggregate = ~19us of saturated drain, so
everything else is organized to start that drain as early as possible
and keep it gap-free.

Key structure:
  * PE outer-product: ex/ey profiles are computed once on 12
    partitions; the idle PE engine broadcasts ey (bf16) to all 128
    partitions (one K=12 matmul per pair against a 0/1 block-selector)
    and each output chunk is produced by exactly ONE DVE/ACT op
    reading PSUM and scaling by the transposed x-profile EXT
    (per-partition scalar) -- the engine minimum per output element.
  * The start-of-kernel all-engine barrier is rebuilt WITHOUT the
    GpSimd/Pool engine (nothing here uses it; constants arrive as
    host-provided DMA inputs, and no float activation biases means no
    const-AP reads -- the only thing the stock barrier protects here).
  * The unused Activation-engine HWDGE queue is dropped from the
    module so the boot-time queue arming has less to do.
  * ACT's Exp table (~1.3us load) is warmed by a dummy Exp on a
    memset tile before the inputs even arrive.
  * The per-axis min of (h-m)^2 is computed EXACTLY from the labels
    alone with the +-2^23 round-to-integer trick + clamp (tiny DVE
    ops hidden under the ACT Squares); the y-side scalars are
    computed first so the y-profile Exp (which gates the matmuls)
    fires as early as possible.
  * Pair 0's three chunks stream out as separate DMAs right behind
    their finals; later pairs go as whole 576KB DMAs.
  * Output rows are staged as h = 3*par + c so each SBUF partition is
    one contiguous 4608B DRAM run.  EXT[par,c,p] = ex_p[3*par+c] comes
    from 3 stride-3 PE transposes.
"""

import numpy as np
from contextlib import ExitStack

import concourse.bacc as bacc
import concourse.bass as bass
import concourse.tile as tile
from concourse import mybir
from concourse.bass_utils import run_bass_kernel_spmd

B, CH, H, W = 16, 3, 384, 384
NCLS = 6
N_CORES = 8
BPC = B // N_CORES            # batches per core = 2
PAIRS = BPC * NCLS            # (b,c) pairs per core = 12
P = 128
C3 = H // P                   # 3 chunks of 128 rows
LN_H = float(np.log(H))
RND = 12582912.0              # 1.5 * 2^23: add+subtract rounds to integer
F32 = mybir.dt.float32
BF16 = mybir.dt.bfloat16
AF = mybir.ActivationFunctionType
ALU = mybir.AluOpType

# packed f32 input layout: [labels(2) | logw(1) | grid(384) | ident(12)]
PK_LAB, PK_LW, PK_GRID, PK_ID = 0, 2, 3, 387
PK_N = PK_ID + PAIRS

# engine for the finals, per pair ('v'=DVE, 'a'=ACT): DVE 20 / ACT 16,
# v-heavy at the tail where ACT is the slower engine.
BALANCE = ["vav", "ava", "vav", "ava", "vav", "ava", "vav", "ava",
           "vav", "vav", "vav", "vav"]


def _barrier_without_pool(self, *, sem_only: bool = False):
    engines = [e for e in self.engines if e != mybir.EngineType.Pool]
    if sem_only:
        for inst in self._sem_only_all_engine_barrier_insts("aeb"):
            self.engines[inst.engine].add_instruction(inst)
    else:
        self.multi_engine_barrier(engines)


def build_bass() -> bass.Bass:
    orig_barrier = bass.Bass.all_engine_barrier
    bass.Bass.all_engine_barrier = _barrier_without_pool
    try:
        nc = bacc.Bacc("TRN2", target_bir_lowering=False, debug=False,
                       num_devices=N_CORES)
        # Drop the Activation HWDGE queue: all DMAs here go through the
        # SP queue (or SWDGE), and fewer declared queues means less
        # boot-time queue arming.
        nc.m.queues = [
            q for q in nc.m.queues
            if not (getattr(q, "is_HWDGE", False)
                    and q.engine == mybir.EngineType.Activation)
        ]
        try:
            nc.hwdge_engines.discard(mybir.EngineType.Activation)
        except Exception:
            pass
        _build_body(nc)
        nc.finalize()
    finally:
        bass.Bass.all_engine_barrier = orig_barrier
    return nc


def _build_body(nc) -> None:
    pack1 = nc.dram_tensor("pack1", [PAIRS, PK_N], F32, kind="ExternalInput")
    bigsld = nc.dram_tensor("bigsld", [PAIRS, PAIRS * P], BF16,
                            kind="ExternalInput")
    out = nc.dram_tensor("out", [PAIRS * H, W], F32, kind="ExternalOutput")

    with ExitStack() as ctx:
        tc = ctx.enter_context(tile.TileContext(nc))
        singles = ctx.enter_context(tc.tile_pool(name="singles", bufs=1))
        psT = ctx.enter_context(tc.tile_pool(name="psT", bufs=2,
                                             space="PSUM"))
        psB = ctx.enter_context(tc.tile_pool(name="psB", bufs=6,
                                             space="PSUM"))
        stage = ctx.enter_context(tc.tile_pool(name="stage", bufs=12))

        # ---- input DMAs on the SP HWDGE queue -----------------------------
        pk = singles.tile([PAIRS, PK_N], F32)
        nc.sync.dma_start(out=pk, in_=pack1[:, :])
        bigsel = singles.tile([PAIRS, PAIRS * P], BF16)
        nc.sync.dma_start(out=bigsel, in_=bigsld[:, :])

        lab = pk[:, PK_LAB:PK_LAB + 2]          # (mx, my)/H in [0,1)
        lwb = pk[:, PK_LW:PK_LW + 1]
        iog = pk[:, PK_GRID:PK_GRID + W]        # j/384 grid
        ident = pk[:, PK_ID:PK_ID + PAIRS]

        # ---- warm the ACT Exp table before inputs arrive ------------------
        zz = singles.tile([PAIRS, 2], F32)
        nc.vector.memset(zz, 0.0)
        warm = singles.tile([PAIRS, 1], F32)
        nc.scalar.activation(out=warm, in_=zz[:, 0:1], func=AF.Exp,
                             bias=zz[:, 1:2], scale=1.0)
        zcol = zz[:, 1:2]                        # zeros bias AP

        # ---- per-pair scalars (DVE, tiny; y-side first) -------------------
        # inv_s = exp(-lw - lnH); grid is j/384 so sq=(lab-j/384)^2 and the
        # 384^2 folds into the exp scales.
        # exact min of (h-m)^2 over integer h in [0,383], from labels only:
        # h* = clamp(round(384*lab), max 383), min = (384*lab-h*)^2 (grid^2)
        nlw = singles.tile([PAIRS, 1], F32)
        nc.vector.tensor_scalar(out=nlw, in0=lwb, scalar1=-1.0,
                                scalar2=-LN_H, op0=ALU.mult, op1=ALU.add)
        inv_s = singles.tile([PAIRS, 1], F32)
        nc.scalar.activation(out=inv_s, in_=nlw, func=AF.Exp,
                             bias=zcol, scale=1.0)
        m2c = singles.tile([PAIRS, 2], F32)     # m = 384*lab  (x|y cols)
        nc.vector.tensor_scalar_mul(out=m2c, in0=lab, scalar1=float(H))
        t1 = singles.tile([PAIRS, 2], F32)
        nc.vector.tensor_scalar_add(out=t1, in0=m2c, scalar1=RND)
        rr = singles.tile([PAIRS, 2], F32)      # round(m) (half-to-even)
        nc.vector.tensor_scalar_add(out=rr, in0=t1, scalar1=-RND)
        rc = singles.tile([PAIRS, 2], F32)      # clamp to grid max
        nc.vector.tensor_scalar_min(out=rc, in0=rr, scalar1=float(H - 1))
        dd = singles.tile([PAIRS, 2], F32)
        nc.vector.tensor_sub(out=dd, in0=m2c, in1=rc)
        mn = singles.tile([PAIRS, 2], F32)
        nc.vector.tensor_mul(out=mn, in0=dd, in1=dd)

        HH = float(H) * float(H)
        scy = singles.tile([PAIRS, 1], F32)     # y exp scale (negative)
        nc.vector.tensor_scalar_mul(out=scy, in0=inv_s, scalar1=-0.025 * HH)
        pscy = singles.tile([PAIRS, 1], F32)
        nc.vector.tensor_scalar_mul(out=pscy, in0=inv_s, scalar1=0.025)
        nby = singles.tile([PAIRS, 1], F32)     # y exp bias >= 0
        nc.vector.tensor_mul(out=nby, in0=mn[:, 1:2], in1=pscy)
        scx = singles.tile([PAIRS, 1], F32)
        nc.vector.tensor_scalar_mul(out=scx, in0=inv_s, scalar1=-0.5 * HH)
        pscx = singles.tile([PAIRS, 1], F32)
        nc.vector.tensor_scalar_mul(out=pscx, in0=inv_s, scalar1=0.5)
        nbx = singles.tile([PAIRS, 1], F32)
        nc.vector.tensor_mul(out=nbx, in0=mn[:, 0:1], in1=pscx)

        # ---- profiles: y first (feeds the matmuls) ------------------------
        sqxy = singles.tile([PAIRS, 2, W], F32)
        nc.scalar.activation(out=sqxy[:, 1, :], in_=iog, func=AF.Square,
                             bias=lab[:, 1:2], scale=-1.0)
        ey = singles.tile([PAIRS, W], BF16)     # y profile (matmul rhs)
        nc.scalar.activation(out=ey, in_=sqxy[:, 1, :], func=AF.Exp,
                             bias=nby, scale=scy)
        nc.scalar.activation(out=sqxy[:, 0, :], in_=iog, func=AF.Square,
                             bias=lab[:, 0:1], scale=-1.0)
        ex = singles.tile([PAIRS, W], F32)      # x profile (to transpose)
        nc.scalar.activation(out=ex, in_=sqxy[:, 0, :], func=AF.Exp,
                             bias=nbx, scale=scx)

        # ---- first matmul can go as soon as ey lands ----------------------
        def pair_matmul(p):
            ps = psB.tile([P, W], F32)
            nc.tensor.matmul(ps, bigsel[:, p * P:(p + 1) * P], ey,
                             start=True, stop=True)
            return ps

        ps0 = pair_matmul(0)

        # ---- EXT[par, c, p] = ex_p[3*par + c] via 3 strided PE transposes -
        exr = ex[:, :].rearrange("p (h c) -> p c h", c=C3)
        ext = singles.tile([P, C3, PAIRS], F32)
        for c in range(C3):
            pt = psT.tile([P, PAIRS], F32)
            nc.tensor.transpose(pt, exr[:, c, :], ident)
            nc.vector.tensor_copy(out=ext[:, c, :], in_=pt)

        # ---- main loop: one final op per output chunk, then one DMA -------
        for p in range(PAIRS):
            ps = ps0 if p == 0 else pair_matmul(p)
            st = stage.tile([P, C3, W], F32)
            od = out[p * H:(p + 1) * H, :].rearrange(
                "(par c) w -> par c w", c=C3)
            for c in range(C3):
                scal = ext[:, c, p:p + 1]
                if BALANCE[p][c] == "v":
                    nc.vector.tensor_scalar_mul(out=st[:, c, :], in0=ps,
                                                scalar1=scal)
                else:
                    nc.scalar.mul(out=st[:, c, :], in_=ps, mul=scal)
                if p == 0:
                    nc.sync.dma_start(out=od[:, c, :], in_=st[:, c, :])
            # DRAM row (within pair p) = 3*par + c: one contiguous 4608B
            # run per partition.
            if p > 0:
                nc.sync.dma_start(out=od, in_=st)


LAST_RESULTS = None  # BassKernelResults of the most recent kernel() call


def _pack_inputs(labels: np.ndarray, log_weight: np.ndarray) -> np.ndarray:
    """[12, 399] per-core f32 pack: labels | logw | grid | identity."""
    pk = np.empty((PAIRS, PK_N), dtype=np.float32)
    pk[:, PK_LAB:PK_LAB + 2] = labels
    pk[:, PK_LW] = np.float32(log_weight).reshape(())
    pk[:, PK_GRID:PK_GRID + W] = (np.arange(W, dtype=np.float32)
                                  / np.float32(W))[None, :]
    pk[:, PK_ID:PK_ID + PAIRS] = np.eye(PAIRS, dtype=np.float32)
    return pk


def kernel(x: np.ndarray, labels: np.ndarray,
           log_weight: np.ndarray, **run_kwargs) -> np.ndarray:
    global LAST_RESULTS
    del x  # only its (hardcoded) shape matters
    import ml_dtypes
    nc = build_bass()
    labels = np.ascontiguousarray(labels, dtype=np.float32)
    bigsel = np.kron(np.eye(PAIRS, dtype=np.float32),
                     np.ones((1, P), dtype=np.float32)
                     ).astype(ml_dtypes.bfloat16)
    in_maps = [
        {
            "pack1": _pack_inputs(
                labels[i * BPC:(i + 1) * BPC].reshape(PAIRS, 2), log_weight),
            "bigsld": bigsel,
        }
        for i in range(N_CORES)
    ]
    res = run_bass_kernel_spmd(nc, in_maps, core_ids=list(range(N_CORES)),
                               **run_kwargs)
    LAST_RESULTS = res
    outs = [r["out"].reshape(BPC, NCLS, H, W) for r in res.results]
    return np.concatenate(outs, axis=0)


if __name__ == "__main__":
    rng = np.random.default_rng(0)
    x = rng.standard_normal((B, CH, H, W), dtype=np.float32)
    labels = rng.random((B, 2 * NCLS), dtype=np.float32)
    lw = rng.random((1, 1, 1, 1), dtype=np.float32)
    y = kernel(x=x, labels=labels, log_weight=lw)
    print(y.shape, y.dtype, y.min(), y.max())


# revision 17
# speedup vs baseline: 1.1204x; 1.0116x over previous
"""Trainium2 Bass kernel for nn_HeatmapLayer: separable Gaussian heatmaps.

Reference math (per batch b, class c):
    mx = labels[b, 2c] * H ; my = labels[b, 2c+1] * W          (H = W = 384)
    sigma = H * exp(log_weight)
    dx2[h] = (h - mx)^2 / sigma        ; normalized by its min over h
    dy2[w] = (w - my)^2 / (20 * sigma) ; normalized by its min over w
    out[b,c,h,w] = exp(-0.5*(dx2[h] + dy2[w])) = ex[h] * ey[w]

Each (b,c) heatmap is a rank-1 outer product of two 384-length
profiles; 2 batches x 6 classes = 12 pairs per core (batch-parallel
over 8 cores).  The kernel is output-DMA-bound: 7.08MB/core over 16
DMA engines at ~360GB/s aggregate = ~19us of saturated drain, so
everything else is organized to start that drain as early as possible
and keep it gap-free.

Key structure:
  * PE outer-product: ex/ey profiles are computed once on 12
    partitions; the idle PE engine broadcasts ey (bf16) to all 128
    partitions (one K=12 matmul per pair against a 0/1 block-selector)
    and each output chunk is produced by exactly ONE DVE/ACT op
    reading PSUM and scaling by the transposed x-profile EXT
    (per-partition scalar) -- the engine minimum per output element.
  * The start-of-kernel all-engine barrier is rebuilt WITHOUT the
    GpSimd/Pool engine (nothing here uses it; constants arrive as
    host-provided DMA inputs, and no float activation biases means no
    const-AP reads -- the only thing the stock barrier protects here).
  * The unused Activation-engine HWDGE queue is dropped from the
    module so the boot-time queue arming has less to do.
  * ACT's Exp table (~1.3us load) is warmed by a dummy Exp on a
    memset tile before the inputs even arrive.
  * The per-axis min of (h-m)^2 is computed EXACTLY from the labels
    alone with the +-2^23 round-to-integer trick + clamp (tiny DVE
    ops hidden under the ACT Squares); the y-side scalars are
    computed first so the y-profile Exp (which gates the matmuls)
    fires as early as possible.
  * Pair 0's three chunks stream out as separate DMAs right behind
    their finals; later pairs go as whole 576KB DMAs.
  * Output rows are staged as h = 3*par + c so each SBUF partition is
    one contiguous 4608B DRAM run.  EXT[par,c,p] = ex_p[3*par+c] comes
    from 3 stride-3 PE transposes.
"""

import numpy as np
from contextlib import ExitStack

import concourse.bacc as bacc
import concourse.bass as bass
import concourse.tile as tile
from concourse import mybir
from concourse.bass_utils import run_bass_kernel_spmd

B, CH, H, W = 16, 3, 384, 384
NCLS = 6
N_CORES = 8
BPC = B // N_CORES            # batches per core = 2
PAIRS = BPC * NCLS            # (b,c) pairs per core = 12
P = 128
C3 = H // P                   # 3 chunks of 128 rows
LN_H = float(np.log(H))
RND = 12582912.0              # 1.5 * 2^23: add+subtract rounds to integer
F32 = mybir.dt.float32
BF16 = mybir.dt.bfloat16
AF = mybir.ActivationFunctionType
ALU = mybir.AluOpType

# packed f32 input layout: [labels(2) | logw(1) | grid(384) | ident(12)]
PK_LAB, PK_LW, PK_GRID, PK_ID = 0, 2, 3, 387
PK_N = PK_ID + PAIRS

# engine for the finals, per pair ('v'=DVE, 'a'=ACT): DVE 20 / ACT 16,
# v-heavy at the tail where ACT is the slower engine.
BALANCE = ["vav", "ava", "vav", "ava", "vav", "ava", "vav", "ava",
           "vav", "vav", "vav", "vav"]


def _barrier_without_pool(self, *, sem_only: bool = False):
    engines = [e for e in self.engines if e != mybir.EngineType.Pool]
    if sem_only:
        for inst in self._sem_only_all_engine_barrier_insts("aeb"):
            self.engines[inst.engine].add_instruction(inst)
    else:
        self.multi_engine_barrier(engines)


def build_bass() -> bass.Bass:
    orig_barrier = bass.Bass.all_engine_barrier
    bass.Bass.all_engine_barrier = _barrier_without_pool
    try:
        nc = bacc.Bacc("TRN2", target_bir_lowering=False, debug=False,
                       num_devices=N_CORES)
        _build_body(nc)
        nc.finalize()
    finally:
        bass.Bass.all_engine_barrier = orig_barrier
    return nc


def _build_body(nc) -> None:
    pack1 = nc.dram_tensor("pack1", [PAIRS, PK_N], F32, kind="ExternalInput")
    bigsld = nc.dram_tensor("bigsld", [PAIRS, PAIRS * P], BF16,
                            kind="ExternalInput")
    out = nc.dram_tensor("out", [PAIRS * H, W], F32, kind="ExternalOutput")

    with ExitStack() as ctx:
        tc = ctx.enter_context(tile.TileContext(nc))
        singles = ctx.enter_context(tc.tile_pool(name="singles", bufs=1))
        psT = ctx.enter_context(tc.tile_pool(name="psT", bufs=2,
                                             space="PSUM"))
        psB = ctx.enter_context(tc.tile_pool(name="psB", bufs=6,
                                             space="PSUM"))
        stage = ctx.enter_context(tc.tile_pool(name="stage", bufs=12))

        # ---- input DMAs, one per HWDGE queue ------------------------------
        pk = singles.tile([PAIRS, PK_N], F32)
        nc.sync.dma_start(out=pk, in_=pack1[:, :])
        bigsel = singles.tile([PAIRS, PAIRS * P], BF16)
        nc.scalar.dma_start(out=bigsel, in_=bigsld[:, :])

        lab = pk[:, PK_LAB:PK_LAB + 2]          # (mx, my)/H in [0,1)
        lwb = pk[:, PK_LW:PK_LW + 1]
        iog = pk[:, PK_GRID:PK_GRID + W]        # j/384 grid
        ident = pk[:, PK_ID:PK_ID + PAIRS]

        # ---- warm the ACT Exp table before inputs arrive ------------------
        zz = singles.tile([PAIRS, 2], F32)
        nc.vector.memset(zz, 0.0)
        warm = singles.tile([PAIRS, 1], F32)
        nc.scalar.activation(out=warm, in_=zz[:, 0:1], func=AF.Exp,
                             bias=zz[:, 1:2], scale=1.0)
        zcol = zz[:, 1:2]                        # zeros bias AP

        # ---- per-pair scalars (DVE, tiny; y-side first) -------------------
        # inv_s = exp(-lw - lnH); grid is j/384 so sq=(lab-j/384)^2 and the
        # 384^2 folds into the exp scales.
        # exact min of (h-m)^2 over integer h in [0,383], from labels only:
        # h* = clamp(round(384*lab), max 383), min = (384*lab-h*)^2 (grid^2)
        nlw = singles.tile([PAIRS, 1], F32)
        nc.vector.tensor_scalar(out=nlw, in0=lwb, scalar1=-1.0,
                                scalar2=-LN_H, op0=ALU.mult, op1=ALU.add)
        inv_s = singles.tile([PAIRS, 1], F32)
        nc.scalar.activation(out=inv_s, in_=nlw, func=AF.Exp,
                             bias=zcol, scale=1.0)
        m2c = singles.tile([PAIRS, 2], F32)     # m = 384*lab  (x|y cols)
        nc.vector.tensor_scalar_mul(out=m2c, in0=lab, scalar1=float(H))
        t1 = singles.tile([PAIRS, 2], F32)
        nc.vector.tensor_scalar_add(out=t1, in0=m2c, scalar1=RND)
        rr = singles.tile([PAIRS, 2], F32)      # round(m) (half-to-even)
        nc.vector.tensor_scalar_add(out=rr, in0=t1, scalar1=-RND)
        rc = singles.tile([PAIRS, 2], F32)      # clamp to grid max
        nc.vector.tensor_scalar_min(out=rc, in0=rr, scalar1=float(H - 1))
        dd = singles.tile([PAIRS, 2], F32)
        nc.vector.tensor_sub(out=dd, in0=m2c, in1=rc)
        mn = singles.tile([PAIRS, 2], F32)
        nc.vector.tensor_mul(out=mn, in0=dd, in1=dd)

        HH = float(H) * float(H)
        scy = singles.tile([PAIRS, 1], F32)     # y exp scale (negative)
        nc.vector.tensor_scalar_mul(out=scy, in0=inv_s, scalar1=-0.025 * HH)
        pscy = singles.tile([PAIRS, 1], F32)
        nc.vector.tensor_scalar_mul(out=pscy, in0=inv_s, scalar1=0.025)
        nby = singles.tile([PAIRS, 1], F32)     # y exp bias >= 0
        nc.vector.tensor_mul(out=nby, in0=mn[:, 1:2], in1=pscy)
        scx = singles.tile([PAIRS, 1], F32)
        nc.vector.tensor_scalar_mul(out=scx, in0=inv_s, scalar1=-0.5 * HH)
        pscx = singles.tile([PAIRS, 1], F32)
        nc.vector.tensor_scalar_mul(out=pscx, in0=inv_s, scalar1=0.5)
        nbx = singles.tile([PAIRS, 1], F32)
        nc.vector.tensor_mul(out=nbx, in0=mn[:, 0:1], in1=pscx)

        # ---- profiles: y first (feeds the matmuls) ------------------------
        sqxy = singles.tile([PAIRS, 2, W], F32)
        nc.scalar.activation(out=sqxy[:, 1, :], in_=iog, func=AF.Square,
                             bias=lab[:, 1:2], scale=-1.0)
        ey = singles.tile([PAIRS, W], BF16)     # y profile (matmul rhs)
        nc.scalar.activation(out=ey, in_=sqxy[:, 1, :], func=AF.Exp,
                             bias=nby, scale=scy)
        nc.scalar.activation(out=sqxy[:, 0, :], in_=iog, func=AF.Square,
                             bias=lab[:, 0:1], scale=-1.0)
        ex = singles.tile([PAIRS, W], F32)      # x profile (to transpose)
        nc.scalar.activation(out=ex, in_=sqxy[:, 0, :], func=AF.Exp,
                             bias=nbx, scale=scx)

        # ---- first matmul can go as soon as ey lands ----------------------
        def pair_matmul(p):
            ps = psB.tile([P, W], F32)
            nc.tensor.matmul(ps, bigsel[:, p * P:(p + 1) * P], ey,
                             start=True, stop=True)
            return ps

        ps0 = pair_matmul(0)

        # ---- EXT[par, c, p] = ex_p[3*par + c] via 3 strided PE transposes -
        exr = ex[:, :].rearrange("p (h c) -> p c h", c=C3)
        ext = singles.tile([P, C3, PAIRS], F32)
        for c in range(C3):
            pt = psT.tile([P, PAIRS], F32)
            nc.tensor.transpose(pt, exr[:, c, :], ident)
            nc.vector.tensor_copy(out=ext[:, c, :], in_=pt)

        # ---- main loop: one final op per output chunk, then one DMA -------
        for p in range(PAIRS):
            ps = ps0 if p == 0 else pair_matmul(p)
            st = stage.tile([P, C3, W], F32)
            od = out[p * H:(p + 1) * H, :].rearrange(
                "(par c) w -> par c w", c=C3)
            for c in range(C3):
                scal = ext[:, c, p:p + 1]
                if BALANCE[p][c] == "v":
                    nc.vector.tensor_scalar_mul(out=st[:, c, :], in0=ps,
                                                scalar1=scal)
                else:
                    nc.scalar.mul(out=st[:, c, :], in_=ps, mul=scal)
                if p == 0:
                    nc.sync.dma_start(out=od[:, c, :], in_=st[:, c, :])
            # DRAM row (within pair p) = 3*par + c: one contiguous 4608B
            # run per partition.
            if p > 0:
                nc.sync.dma_start(out=od, in_=st)


LAST_RESULTS = None  # BassKernelResults of the most recent kernel() call


def _pack_inputs(labels: np.ndarray, log_weight: np.ndarray) -> np.ndarray:
    """[12, 399] per-core f32 pack: labels | logw | grid | identity."""
    pk = np.empty((PAIRS, PK_N), dtype=np.float32)
    pk[:, PK_LAB:PK_LAB + 2] = labels
    pk[:, PK_LW] = np.float32(log_weight).reshape(())
    pk[:, PK_GRID:PK_GRID + W] = (np.arange(W, dtype=np.float32)
                                  / np.float32(W))[None, :]
    pk[:, PK_ID:PK_ID + PAIRS] = np.eye(PAIRS, dtype=np.float32)
    return pk


def kernel(x: np.ndarray, labels: np.ndarray,
           log_weight: np.ndarray, **run_kwargs) -> np.ndarray:
    global LAST_RESULTS
    del x  # only its (hardcoded) shape matters
    import ml_dtypes
    nc = build_bass()
    labels = np.ascontiguousarray(labels, dtype=np.float32)
    bigsel = np.kron(np.eye(PAIRS, dtype=np.float32),
                     np.ones((1, P), dtype=np.float32)
                     ).astype(ml_dtypes.bfloat16)
    in_maps = [
        {
            "pack1": _pack_inputs(
                labels[i * BPC:(i + 1) * BPC].reshape(PAIRS, 2), log_weight),
            "bigsld": bigsel,
        }
        for i in range(N_CORES)
    ]
    res = run_bass_kernel_spmd(nc, in_maps, core_ids=list(range(N_CORES)),
                               **run_kwargs)
    LAST_RESULTS = res
    outs = [r["out"].reshape(BPC, NCLS, H, W) for r in res.results]
    return np.concatenate(outs, axis=0)


if __name__ == "__main__":
    rng = np.random.default_rng(0)
    x = rng.standard_normal((B, CH, H, W), dtype=np.float32)
    labels = rng.random((B, 2 * NCLS), dtype=np.float32)
    lw = rng.random((1, 1, 1, 1), dtype=np.float32)
    y = kernel(x=x, labels=labels, log_weight=lw)
    print(y.shape, y.dtype, y.min(), y.max())
